# revision 20
# baseline (speedup 1.0000x reference)
"""GRU decoder kernel for Trainium2 (Bass/Tile), data-parallel over batch N.

Problem: T=1000, N=64, H=1024, C=256 GRU with batched input projection and
log_softmax output.  Each of the 8 cores handles N/8 = 8 batch rows.

v2 design (vs v1's 3 serial phases):
  - The input projection (enc @ W_ih.T) is interleaved INTO the GRU loop:
    block k+1 is projected (DMA / bf16 convert / PE transpose / matmul /
    bias-copy into SBUF rings) while block k's GRU steps run, filling the
    idle windows of the latency-bound recurrence.  xp never round-trips
    through DRAM.  The For_i body covers 2 blocks (64 steps) so the xp
    rings double-buffer by block parity with no WAR stalls.
  - The 8 batch rows run as 2 independent 4-row chains whose dependency
    chains interleave, converting the recurrence from latency-bound
    toward engine-throughput-bound.
  - r and z share one PSUM region and ONE sigmoid per chain/step: the
    z gate's weights/biases are sign-flipped on the host so
    sigmoid(-(xz+hz)) = 1-z comes out of the same instruction as r.
    The n-gate b_hh bias is baked into the xp ring so each chain/step
    does a single PSUM-group-opening preload matmul (PSUM start=True
    clears the whole bank, so each bank holds exactly one group:
    4 banks = 2 chains x 2 step-parity).
  - h' = (1-z)*n + z*h via zn + t2 with q,t2 on gpsimd in the tanh window.
  - h stays bf16 end-to-end; flushed to DRAM hist by pure DMA.
  - log_softmax is a compact single-pass phase 3: h = tanh(..) is bounded
    so no max pass is needed; exp+accum per 16-step block, one Ln, then
    subtract + store.  Keeps exp/ln ACT table loads out of the hot loop
    (sigmoid/tanh share one table set; exp/ln do not).
"""

import sys

for _p in ("/opt/trn_rl_repo",):
    if _p not in sys.path:
        sys.path.insert(0, _p)

import numpy as np
import ml_dtypes

import concourse.bass as bass
import concourse.bacc as bacc
import concourse.mybir as mybir
import concourse.tile as tile
from concourse.bass import ds, ts
from concourse.bass_utils import run_bass_kernel_spmd

F32 = mybir.dt.float32
BF16 = mybir.dt.bfloat16
AF = mybir.ActivationFunctionType
OP = mybir.AluOpType

T, N, H, C = 1000, 64, 1024, 256
G3 = 3 * C  # 768
NCORES = 8
NB = N // NCORES  # 8 batch rows per core
NBC = NB // 2     # 4 batch rows per chain
KH = H // 128     # 8 k-chunks for projection
M3 = G3 // 128    # 6 m-chunks of gate dim
KC = C // 128     # 2 k-chunks for recurrence
U = 32            # projection block size; For_i body covers 2 blocks
U2 = 2 * U
DEBUG_HIST = False  # expose h history as an extra output (debug only)


def build_gru_nc(t_total=T):
    nc = bacc.Bacc(None, target_bir_lowering=False)

    nblocks = (t_total + U - 1) // U
    tail_t = t_total - U * (nblocks - 1)  # 1..32 steps in the last block
    tpad = nblocks * U
    # loop iterations cover block pairs (2k, 2k+1); must leave >= 1 block
    # for the tail.  n_pairs pairs -> gru blocks 0..2*n_pairs-1, proj
    # blocks 1..2*n_pairs.
    n_pairs = max(0, (nblocks - 1) // 2)

    # ---- parameters -----------------------------------------------------
    enc = nc.declare_dram_parameter("enc", [tpad * NB, H], F32, isOutput=False)
    # W_ih as lhsT tiles: wih[p, k, m*128+q] = W_ih[m*128+q, k*128+p]
    # (z-gate rows sign-flipped on host)
    wih = nc.declare_dram_parameter("wih", [128, KH, G3], BF16, isOutput=False)
    whh = nc.declare_dram_parameter("whh", [128, KC, G3], BF16, isOutput=False)
    idt_b = nc.declare_dram_parameter("idt_b", [128, 128], BF16, isOutput=False)
    # bias column m: rz chunks get +-(b_ih+b_hh), n chunks get b_ih only
    biasc = nc.declare_dram_parameter("biasc", [128, M3], F32, isOutput=False)
    # n-gate b_hh broadcast to every ring slot: [p, slot, c2, b]
    bhnb = nc.declare_dram_parameter(
        "bhnb", [128, U2 * 2 * NB], BF16, isOutput=False
    )
    out = nc.declare_dram_parameter("out", [t_total * NB, C], F32, isOutput=True)

    # ---- DRAM scratch ---------------------------------------------------
    # h history, row(t,b)-major per c2 chunk: affine in the row-unit iv
    if DEBUG_HIST:
        hist = nc.declare_dram_parameter(
            "hist", [128, KC, t_total * NB], BF16, isOutput=True
        )
    else:
        hist = nc.dram_tensor("hist", [128, KC, t_total * NB], BF16)

    with tile.TileContext(nc) as tc:
        with (
            tc.tile_pool(name="const", bufs=1) as cpool,
            tc.tile_pool(name="work", bufs=2) as wpool,
        ):
            # PSUM pools are entered manually so they can be closed before
            # phase 3 (8-bank budget).  All PSUM tiles are full banks.
            pp_ctx = tc.tile_pool(name="projps", bufs=1, space="PSUM")
            ppool = pp_ctx.__enter__()
            tp_ctx = tc.tile_pool(name="tpps", bufs=2, space="PSUM")
            tpool = tp_ctx.__enter__()
            gp_ctx = tc.tile_pool(name="grups", bufs=1, space="PSUM")
            gpool = gp_ctx.__enter__()

            idt_b_sb = cpool.tile([128, 128], BF16)
            nc.sync.dma_start(idt_b_sb, idt_b[:, :])
            wih_sb = cpool.tile([128, KH, G3], BF16)
            nc.sync.dma_start(wih_sb, wih[:, :, :])
            whh_sb = cpool.tile([128, KC, G3], BF16)
            nc.sync.dma_start(whh_sb, whh[:, :, :])
            biasc_sb = cpool.tile([128, M3], F32)
            nc.sync.dma_start(biasc_sb, biasc[:, :])

            # persistent rings, double-length (2 blocks by parity)
            # pre_ring chunks 0..3 = xp r|z (proj-written), 4..5 = b_hh n
            # gate broadcast (DMA'd once, never rewritten)
            pre_ring = cpool.tile([128, U2, 6, NB], BF16)
            n_ring = cpool.tile([128, U2, 2, NB], F32)
            h_ring = cpool.tile([128, KC, U2, NB], BF16)
            encT = cpool.tile([128, KH, 2 * 128], BF16)

            nc.sync.dma_start(
                pre_ring[:, :, 4:6, :],
                bhnb[:, :].rearrange("p (s c b) -> p s c b", s=U2, c=2),
            )
            nc.vector.memset(h_ring[:, :, U2 - 1, :], 0.0)

            # pre-warm the sigmoid/tanh table set
            warm = wpool.tile([1, 1], F32, tag="warm")
            nc.scalar.activation(warm, biasc_sb[:1, :1], AF.Sigmoid)

            # proj psum: 2 full banks, m-major sequential groups ping-pong
            psm = [
                ppool.tile([128, 512], F32, tag=f"pj{j}", name=f"pj{j}")
                for j in range(2)
            ]
            # gru psum: one full bank per (chain, step-parity)
            ps_cp = [
                [
                    gpool.tile([128, 512], F32, tag=f"g{c}{p}", name=f"g{c}{p}")
                    for p in range(2)
                ]
                for c in range(2)
            ]

            # ============ projection pieces (one U-block) ================
            # base: DRAM row offset; sbase: ring slot base (0 or U).
            # Returns U piece-lists to interleave, one per emission step.
            def proj_pieces(base, sbase):
                pieces = [[] for _ in range(U)]
                enc_f = [None, None]
                enc_b = [None, None]

                def dma_rt(rt):
                    def f():
                        enc_f[rt] = wpool.tile(
                            [128, H], F32, tag=f"enc_in{rt}", bufs=2,
                            name=f"enc_in{rt}",
                        )
                        nc.sync.dma_start(
                            enc_f[rt], enc[ds(base + rt * 128, 128), :]
                        )
                    return f

                def conv_rt(rt):
                    def f():
                        enc_b[rt] = wpool.tile(
                            [128, H], BF16, tag=f"enc_bf{rt}", bufs=2,
                            name=f"enc_bf{rt}",
                        )
                        nc.gpsimd.tensor_copy(enc_b[rt], enc_f[rt])
                    return f

                def tpose(k):
                    def f():
                        for rt in range(2):
                            ps_t = tpool.tile([128, 1024], BF16, tag="tpose")
                            nc.tensor.transpose(
                                ps_t[:, :128], enc_b[rt][:, ts(k, 128)],
                                idt_b_sb,
                            )
                            eng = nc.vector if (k + rt) % 2 == 0 else nc.scalar
                            if eng is nc.vector:
                                nc.vector.tensor_copy(
                                    encT[:, k, ds(rt * 128, 128)],
                                    ps_t[:, :128],
                                )
                            else:
                                nc.scalar.activation(
                                    encT[:, k, ds(rt * 128, 128)],
                                    ps_t[:, :128], AF.Copy,
                                )
                    return f

                def mm(m, ks):
                    def f():
                        for k in ks:
                            nc.tensor.matmul(
                                psm[m % 2][:, : U * NB],
                                lhsT=wih_sb[:, k, ts(m, 128)],
                                rhs=encT[:, k, :],
                                start=(k == 0),
                                stop=(k == KH - 1),
                            )
                    return f

                def copy_m(m):
                    def f():
                        src = psm[m % 2][:, : U * NB].rearrange(
                            "p (t j) -> p t j", j=NB
                        )
                        if m < 4:
                            nc.scalar.activation(
                                pre_ring[:, sbase : sbase + U, m, :], src,
                                AF.Identity, bias=biasc_sb[:, m : m + 1],
                            )
                        else:
                            nc.vector.tensor_scalar(
                                n_ring[:, sbase : sbase + U, m - 4, :], src,
                                biasc_sb[:, m : m + 1], None, OP.add,
                            )
                    return f

                pieces[0].append(dma_rt(0))
                pieces[1].append(dma_rt(1))
                pieces[2].append(conv_rt(0))
                pieces[3].append(conv_rt(1))
                for k in range(KH):
                    pieces[4 + k].append(tpose(k))
                # m-major groups, 3 mms/step over s=12..27; copy after stop
                for m in range(M3):
                    s0 = 12 + 8 * m // 3
                    pieces[s0].append(mm(m, (0, 1, 2)))
                    pieces[s0 + 1].append(mm(m, (3, 4, 5)))
                    pieces[s0 + 2].append(mm(m, (6, 7)))
                    pieces[s0 + 2].append(copy_m(m))
                return pieces

            # ================= GRU step =================================
            def emit_step(s, c):
                pv = (s - 1) % U2
                b0 = c * NBC
                bs = slice(b0, b0 + NBC)
                ps = ps_cp[c][s % 2][:, : 6 * NBC].rearrange(
                    "p (g j) -> p g j", g=6
                )
                ps_rz = ps[:, 0:4, :]
                ps_n = ps[:, 4:6, :]

                # single group-opening preload: xp r|z plus n-gate b_hh
                nc.tensor.matmul(
                    ps, lhsT=idt_b_sb, rhs=pre_ring[:, s, :, bs],
                    start=True, stop=False,
                )
                # W_hh matmuls: rz gates first (release the sigmoid), n after
                for i, m in enumerate((0, 1, 2, 3, 4, 5)):
                    for k in range(KC):
                        nc.tensor.matmul(
                            ps[:, i, :],
                            lhsT=whh_sb[:, k, ts(m, 128)],
                            rhs=h_ring[:, k, pv, bs],
                            start=False,
                            stop=(m == 5 and k == KC - 1),
                        )

                s_rz = wpool.tile([128, 4, NBC], F32, tag=f"s_rz{c}")
                nc.scalar.activation(s_rz, ps_rz, AF.Sigmoid)
                r = s_rz[:, 0:2, :]
                zb = s_rz[:, 2:4, :]
                hp = h_ring[:, :, pv, bs]

                # vector: m1 x2, nin(c0), zn x2, h' x2; gpsimd: nin(c1),
                # q x2, t2 x2 (gpsimd cannot touch PSUM)
                m1 = wpool.tile([128, 2, NBC], F32, tag=f"m1_{c}")
                nc.vector.tensor_tensor(m1, r, ps_n, OP.mult)
                nin = wpool.tile([128, 2, NBC], F32, tag=f"nin{c}")
                e_nin = nc.vector if c == 0 else nc.gpsimd
                e_nin.tensor_tensor(nin, m1, n_ring[:, s, :, bs], OP.add)
                n_sb = wpool.tile([128, 2, NBC], F32, tag=f"n_sb{c}")
                nc.scalar.activation(n_sb, nin, AF.Tanh)

                q = wpool.tile([128, 2, NBC], F32, tag=f"q_{c}")
                t2 = wpool.tile([128, 2, NBC], F32, tag=f"t2_{c}")
                nc.gpsimd.tensor_tensor(q, zb, hp, OP.mult)
                nc.gpsimd.tensor_tensor(t2, hp, q, OP.subtract)

                zn = wpool.tile([128, 2, NBC], F32, tag=f"zn{c}")
                nc.vector.tensor_tensor(zn, zb, n_sb, OP.mult)
                nc.vector.tensor_tensor(h_ring[:, :, s, bs], zn, t2, OP.add)

            def flush(rowbase, lo, hi):
                for c2 in range(KC):
                    nc.sync.dma_start(
                        hist[:, c2, ds(rowbase + lo * NB, (hi - lo) * NB)],
                        h_ring[:, c2, lo:hi, :].rearrange("p t b -> p (t b)"),
                    )

            # ================= prologue: project block 0 =================
            for plist in proj_pieces(0, 0):
                for f in plist:
                    f()

            # ================= main loop (2 blocks / iter) ==============
            if n_pairs > 0:
                with tc.For_i(
                    0, n_pairs * U2 * NB, step=U2 * NB, staggered_reset=True,
                    hint_engines=(mybir.EngineType.PE,),
                ) as iv:
                    pieces_b = proj_pieces(iv + U * NB, U)       # block 2k+1
                    pieces_a = proj_pieces(iv + U2 * NB, 0)      # block 2k+2
                    for s in range(U2):
                        emit_step(s, 0)
                        emit_step(s, 1)
                        if s < U:
                            for f in pieces_b[s]:
                                f()
                        else:
                            for f in pieces_a[s - U]:
                                f()
                        if s in (17, 33, 49):
                            lo = ((s - 1) // 16 - 1) * 16  # 0, 16, 32
                            flush(iv, lo, lo + 16)
                    flush(iv, U2 - 16, U2)

            # ================= tail blocks ==============================
            done_b = 2 * n_pairs  # gru blocks completed by the loop
            rowbase = done_b * U * NB
            # project remaining blocks done_b+1 .. nblocks-1 (block done_b
            # was projected by the loop's last iteration or the prologue)
            rem_proj = []
            for bkt in range(done_b + 1, nblocks):
                rem_proj.append(proj_pieces(bkt * U * NB, (bkt % 2) * U))
            assert len(rem_proj) <= 1, "tail can only project one block"
            rem_steps = t_total - done_b * U

            for s in range(rem_steps):
                sl = (done_b * U + s) % U2
                emit_step(sl, 0)
                emit_step(sl, 1)
                if s < U:
                    for pl in rem_proj:
                        for f in pl[s]:
                            f()
            # flush the tail in 16-step chunks
            sbase0 = (done_b * U) % U2
            off = 0
            while off < rem_steps:
                hi = min(off + 16, rem_steps)
                flush(rowbase, sbase0 + off, sbase0 + hi)
                off = hi

            # close PSUM pools before phase 3
            gp_ctx.__exit__(None, None, None)
            tp_ctx.__exit__(None, None, None)
            pp_ctx.__exit__(None, None, None)

            # ================= phase 3: log_softmax =====================
            # h = tanh(.) is in (-1,1): exp can't overflow, skip max pass.
            with tc.tile_pool(name="p3ps", bufs=2, space="PSUM") as p3pool:
                nblk = (t_total + 15) // 16
                x_all = cpool.tile([128, nblk, C], BF16)
                se_all = cpool.tile([128, nblk], F32)
                s_all = cpool.tile([128, nblk], F32)
                nc.vector.memset(se_all, 1.0)

                blocks = []
                t0 = 0
                b = 0
                while t0 < t_total:
                    bt = min(16, t_total - t0)
                    rows = bt * NB
                    blocks.append((t0, bt, b, rows))
                    hsb = wpool.tile([128, KC, 16, NB], BF16, tag="hsb", bufs=3)
                    nc.sync.dma_start(
                        hsb[:, :, :bt, :].rearrange("p c t b -> p c (t b)"),
                        hist[:, :, ds(t0 * NB, rows)],
                    )
                    ps3 = p3pool.tile([128, 1024], BF16, tag="ps3")
                    for c2 in range(KC):
                        nc.tensor.transpose(
                            ps3[:rows, ts(c2, 128)], hsb[:, c2, :bt, :],
                            idt_b_sb,
                        )
                    nc.vector.tensor_copy(x_all[:rows, b, :], ps3[:rows, :C])
                    escr = wpool.tile([128, C], BF16, tag="escr")
                    nc.scalar.activation(
                        escr[:rows, :], ps3[:rows, :C], AF.Exp,
                        accum_out=se_all[:rows, b : b + 1],
                    )
                    t0 += bt
                    b += 1

                nc.scalar.activation(s_all, se_all, AF.Ln)

                for t0, bt, b, rows in blocks:
                    o_sb = wpool.tile([128, C], F32, tag="o_sb", bufs=3)
                    nc.vector.tensor_scalar(
                        o_sb[:rows, :], x_all[:rows, b, :],
                        s_all[:rows, b : b + 1], None, OP.subtract,
                    )
                    nc.sync.dma_start(out[ds(t0 * NB, rows), :], o_sb[:rows, :])

    nc.compile()
    return nc


def _prep_weights(W_ih, W_hh, b_ih, b_hh):
    bf = ml_dtypes.bfloat16
    # z-gate sign flip: sigmoid(-(xz+hz)) = 1-z from the shared r|z sigmoid
    W_ih = W_ih.copy()
    W_hh = W_hh.copy()
    W_ih[C : 2 * C] *= -1.0
    W_hh[C : 2 * C] *= -1.0

    # lhsT layouts: w[p, k, m*128+q] = W[m*128+q, k*128+p]
    def lhst(W, kc):
        t = W.T.reshape(kc, 128, W.shape[0])  # [k, p, g]
        return np.ascontiguousarray(t.transpose(1, 0, 2)).astype(bf)

    wih = lhst(W_ih, KH)
    whh = lhst(W_hh, KC)
    idt_b = np.eye(128, dtype=bf)
    ball = (b_ih + b_hh).astype(np.float32).copy()
    ball[C : 2 * C] *= -1.0            # z gate: negated sum
    ball[2 * C :] = b_ih[2 * C :]      # n gate: b_ih only
    biasc = np.ascontiguousarray(ball.reshape(M3, 128).T).astype(np.float32)
    # n-gate b_hh broadcast to all U2 ring slots: [p, slot, c2, b]
    bhn_v = b_hh[2 * C :].reshape(2, 128).T.astype(np.float32)  # [p, c2]
    bhnb = np.broadcast_to(
        bhn_v[:, None, :, None], (128, U2, 2, NB)
    ).reshape(128, U2 * 2 * NB).astype(bf)
    bhnb = np.ascontiguousarray(bhnb)
    return wih, whh, idt_b, biasc, bhnb


_CACHED = {}


def _make_in_maps(inputs):
    encoder_output = np.asarray(inputs["encoder_output"], dtype=np.float32)
    W_ih = np.asarray(inputs["W_ih"], dtype=np.float32)
    W_hh = np.asarray(inputs["W_hh"], dtype=np.float32)
    b_ih = np.asarray(inputs["b_ih"], dtype=np.float32)
    b_hh = np.asarray(inputs["b_hh"], dtype=np.float32)
    t_total = encoder_output.shape[0]
    nblocks = (t_total + U - 1) // U
    tpad = nblocks * U
    wih, whh, idt_b, biasc, bhnb = _prep_weights(W_ih, W_hh, b_ih, b_hh)
    in_maps = []
    for c in range(NCORES):
        shard = encoder_output[:, c * NB : (c + 1) * NB, :]  # [T, NB, H]
        enc = np.zeros((tpad * NB, H), dtype=np.float32)
        enc[: t_total * NB] = shard.reshape(t_total * NB, H)
        in_maps.append(
            {
                "enc": enc, "wih": wih, "whh": whh, "idt_b": idt_b,
                "biasc": biasc, "bhnb": bhnb,
            }
        )
    return in_maps


def kernel(encoder_output, W_ih, W_hh, b_ih, b_hh):
    encoder_output = np.asarray(encoder_output, dtype=np.float32)

    t_total = encoder_output.shape[0]
    if "nc" not in _CACHED or _CACHED.get("t") != t_total:
        _CACHED["nc"] = build_gru_nc(t_total)
        _CACHED["t"] = t_total

    in_maps = _make_in_maps(
        {
            "encoder_output": encoder_output,
            "W_ih": W_ih, "W_hh": W_hh, "b_ih": b_ih, "b_hh": b_hh,
        }
    )

    res = run_bass_kernel_spmd(_CACHED["nc"], in_maps, list(range(NCORES)))
    outs = [
        res.results[c]["out"].reshape(t_total, NB, C) for c in range(NCORES)
    ]
    return np.concatenate(outs, axis=1)


if __name__ == "__main__":
    # quick smoke test with small T
    t_small = int(sys.argv[1]) if len(sys.argv) > 1 else 72
    rng = np.random.default_rng(0)
    enc = rng.standard_normal((t_small, N, H), dtype=np.float32)
    s = 0.05
    Wih = rng.standard_normal((G3, H), dtype=np.float32) * s
    Whh = rng.standard_normal((G3, C), dtype=np.float32) * s
    bih = rng.standard_normal(G3).astype(np.float32) * s
    bhh = rng.standard_normal(G3).astype(np.float32) * s

    got = kernel(enc, Wih, Whh, bih, bhh)

    xp = enc.reshape(-1, H) @ Wih.T + bih
    xp = xp.reshape(t_small, N, G3)
    h = np.zeros((N, C), dtype=np.float32)
    outs = []
    sig = lambda x: 1.0 / (1.0 + np.exp(-x))
    for t in range(t_small):
        gh = h @ Whh.T + bhh
        xr, xz, xn = np.split(xp[t], 3, axis=-1)
        hr, hz, hn = np.split(gh, 3, axis=-1)
        r = sig(xr + hr)
        z = sig(xz + hz)
        n = np.tanh(xn + r * hn)
        h = (1.0 - z) * n + z * h
        outs.append(h.copy())
    ref = np.stack(outs)
    mx = ref.max(-1, keepdims=True)
    ref = ref - mx - np.log(np.exp(ref - mx).sum(-1, keepdims=True))

    err = np.abs(got - ref)
    print("abs err max:", err.max(), " rel:", err.max() / np.abs(ref).max())


# revision 22
# speedup vs baseline: 1.0005x; 1.0005x over previous
"""GRU decoder kernel for Trainium2 (Bass/Tile), data-parallel over batch N.

Problem: T=1000, N=64, H=1024, C=256 GRU with batched input projection and
log_softmax output.  Each of the 8 cores handles N/8 = 8 batch rows.

v2 design (vs v1's 3 serial phases):
  - The input projection (enc @ W_ih.T) is interleaved INTO the GRU loop:
    block k+1 is projected (DMA / bf16 convert / PE transpose / matmul /
    bias-copy into SBUF rings) while block k's GRU steps run, filling the
    idle windows of the latency-bound recurrence.  xp never round-trips
    through DRAM.  The For_i body covers 2 blocks (64 steps) so the xp
    rings double-buffer by block parity with no WAR stalls.
  - The 8 batch rows run as 2 independent 4-row chains whose dependency
    chains interleave, converting the recurrence from latency-bound
    toward engine-throughput-bound.
  - r and z share one PSUM region and ONE sigmoid per chain/step: the
    z gate's weights/biases are sign-flipped on the host so
    sigmoid(-(xz+hz)) = 1-z comes out of the same instruction as r.
    The n-gate b_hh bias is baked into the xp ring so each chain/step
    does a single PSUM-group-opening preload matmul (PSUM start=True
    clears the whole bank, so each bank holds exactly one group:
    4 banks = 2 chains x 2 step-parity).
  - h' = (1-z)*n + z*h via zn + t2 with q,t2 on gpsimd in the tanh window.
  - h stays bf16 end-to-end; flushed to DRAM hist by pure DMA.
  - log_softmax is a compact single-pass phase 3: h = tanh(..) is bounded
    so no max pass is needed; exp+accum per 16-step block, one Ln, then
    subtract + store.  Keeps exp/ln ACT table loads out of the hot loop
    (sigmoid/tanh share one table set; exp/ln do not).
"""

import sys

for _p in ("/opt/trn_rl_repo",):
    if _p not in sys.path:
        sys.path.insert(0, _p)

import numpy as np
import ml_dtypes

import concourse.bass as bass
import concourse.bacc as bacc
import concourse.mybir as mybir
import concourse.tile as tile
from concourse.bass import ds, ts
from concourse.bass_utils import run_bass_kernel_spmd

F32 = mybir.dt.float32
BF16 = mybir.dt.bfloat16
AF = mybir.ActivationFunctionType
OP = mybir.AluOpType

T, N, H, C = 1000, 64, 1024, 256
G3 = 3 * C  # 768
NCORES = 8
NB = N // NCORES  # 8 batch rows per core
NBC = NB // 2     # 4 batch rows per chain
KH = H // 128     # 8 k-chunks for projection
M3 = G3 // 128    # 6 m-chunks of gate dim
KC = C // 128     # 2 k-chunks for recurrence
U = 32            # projection block size; For_i body covers 2 blocks
U2 = 2 * U
DEBUG_HIST = False  # expose h history as an extra output (debug only)


def build_gru_nc(t_total=T):
    nc = bacc.Bacc(None, target_bir_lowering=False)

    nblocks = (t_total + U - 1) // U
    tail_t = t_total - U * (nblocks - 1)  # 1..32 steps in the last block
    tpad = nblocks * U
    # loop iterations cover block pairs (2k, 2k+1); must leave >= 1 block
    # for the tail.  n_pairs pairs -> gru blocks 0..2*n_pairs-1, proj
    # blocks 1..2*n_pairs.
    n_pairs = max(0, (nblocks - 1) // 2)

    # ---- parameters -----------------------------------------------------
    enc = nc.declare_dram_parameter("enc", [tpad * NB, H], F32, isOutput=False)
    # W_ih as lhsT tiles: wih[p, k, m*128+q] = W_ih[m*128+q, k*128+p]
    # (z-gate rows sign-flipped on host)
    wih = nc.declare_dram_parameter("wih", [128, KH, G3], BF16, isOutput=False)
    whh = nc.declare_dram_parameter("whh", [128, KC, G3], BF16, isOutput=False)
    idt_b = nc.declare_dram_parameter("idt_b", [128, 128], BF16, isOutput=False)
    # bias column m: rz chunks get +-(b_ih+b_hh), n chunks get b_ih only
    biasc = nc.declare_dram_parameter("biasc", [128, M3], F32, isOutput=False)
    # n-gate b_hh broadcast to every ring slot: [p, slot, c2, b]
    bhnb = nc.declare_dram_parameter(
        "bhnb", [128, U2 * 2 * NB], BF16, isOutput=False
    )
    out = nc.declare_dram_parameter("out", [t_total * NB, C], F32, isOutput=True)

    # ---- DRAM scratch ---------------------------------------------------
    # h history, row(t,b)-major per c2 chunk: affine in the row-unit iv
    if DEBUG_HIST:
        hist = nc.declare_dram_parameter(
            "hist", [128, KC, t_total * NB], BF16, isOutput=True
        )
    else:
        hist = nc.dram_tensor("hist", [128, KC, t_total * NB], BF16)

    with tile.TileContext(nc) as tc:
        with (
            tc.tile_pool(name="const", bufs=1) as cpool,
            tc.tile_pool(name="work", bufs=2) as wpool,
        ):
            # PSUM pools are entered manually so they can be closed before
            # phase 3 (8-bank budget).  All PSUM tiles are full banks.
            pp_ctx = tc.tile_pool(name="projps", bufs=1, space="PSUM")
            ppool = pp_ctx.__enter__()
            tp_ctx = tc.tile_pool(name="tpps", bufs=2, space="PSUM")
            tpool = tp_ctx.__enter__()
            gp_ctx = tc.tile_pool(name="grups", bufs=1, space="PSUM")
            gpool = gp_ctx.__enter__()

            idt_b_sb = cpool.tile([128, 128], BF16)
            nc.sync.dma_start(idt_b_sb, idt_b[:, :])
            wih_sb = cpool.tile([128, KH, G3], BF16)
            nc.sync.dma_start(wih_sb, wih[:, :, :])
            whh_sb = cpool.tile([128, KC, G3], BF16)
            nc.sync.dma_start(whh_sb, whh[:, :, :])
            biasc_sb = cpool.tile([128, M3], F32)
            nc.sync.dma_start(biasc_sb, biasc[:, :])

            # persistent rings, double-length (2 blocks by parity)
            # pre_ring chunks 0..3 = xp r|z (proj-written), 4..5 = b_hh n
            # gate broadcast (DMA'd once, never rewritten)
            pre_ring = cpool.tile([128, U2, 6, NB], BF16)
            n_ring = cpool.tile([128, U2, 2, NB], F32)
            h_ring = cpool.tile([128, KC, U2, NB], BF16)
            encT = cpool.tile([128, KH, 2 * 128], BF16)

            nc.sync.dma_start(
                pre_ring[:, :, 4:6, :],
                bhnb[:, :].rearrange("p (s c b) -> p s c b", s=U2, c=2),
            )
            nc.vector.memset(h_ring[:, :, U2 - 1, :], 0.0)

            # pre-warm the sigmoid/tanh table set
            warm = wpool.tile([1, 1], F32, tag="warm")
            nc.scalar.activation(warm, biasc_sb[:1, :1], AF.Sigmoid)

            # proj psum: 2 full banks, m-major sequential groups ping-pong
            psm = [
                ppool.tile([128, 512], F32, tag=f"pj{j}", name=f"pj{j}")
                for j in range(2)
            ]
            # gru psum: one full bank per (chain, step-parity)
            ps_cp = [
                [
                    gpool.tile([128, 512], F32, tag=f"g{c}{p}", name=f"g{c}{p}")
                    for p in range(2)
                ]
                for c in range(2)
            ]

            # ============ projection pieces (one U-block) ================
            # base: DRAM row offset; sbase: ring slot base (0 or U).
            # Returns U piece-lists to interleave, one per emission step.
            def proj_pieces(base, sbase):
                pieces = [[] for _ in range(U)]
                enc_f = [None, None]
                enc_b = [None, None]

                def dma_rt(rt):
                    def f():
                        enc_f[rt] = wpool.tile(
                            [128, H], F32, tag=f"enc_in{rt}", bufs=2,
                            name=f"enc_in{rt}",
                        )
                        nc.sync.dma_start(
                            enc_f[rt], enc[ds(base + rt * 128, 128), :]
                        )
                    return f

                def conv_rt(rt):
                    def f():
                        enc_b[rt] = wpool.tile(
                            [128, H], BF16, tag=f"enc_bf{rt}", bufs=2,
                            name=f"enc_bf{rt}",
                        )
                        nc.gpsimd.tensor_copy(enc_b[rt], enc_f[rt])
                    return f

                def tpose(k):
                    def f():
                        for rt in range(2):
                            ps_t = tpool.tile([128, 1024], BF16, tag="tpose")
                            nc.tensor.transpose(
                                ps_t[:, :128], enc_b[rt][:, ts(k, 128)],
                                idt_b_sb,
                            )
                            eng = nc.vector if (k + rt) % 2 == 0 else nc.scalar
                            if eng is nc.vector:
                                nc.vector.tensor_copy(
                                    encT[:, k, ds(rt * 128, 128)],
                                    ps_t[:, :128],
                                )
                            else:
                                nc.scalar.activation(
                                    encT[:, k, ds(rt * 128, 128)],
                                    ps_t[:, :128], AF.Copy,
                                )
                    return f

                def mm(m, ks):
                    def f():
                        for k in ks:
                            nc.tensor.matmul(
                                psm[m % 2][:, : U * NB],
                                lhsT=wih_sb[:, k, ts(m, 128)],
                                rhs=encT[:, k, :],
                                start=(k == 0),
                                stop=(k == KH - 1),
                            )
                    return f

                def copy_m(m):
                    def f():
                        src = psm[m % 2][:, : U * NB].rearrange(
                            "p (t j) -> p t j", j=NB
                        )
                        if m < 4:
                            nc.scalar.activation(
                                pre_ring[:, sbase : sbase + U, m, :], src,
                                AF.Identity, bias=biasc_sb[:, m : m + 1],
                            )
                        else:
                            nc.vector.tensor_scalar(
                                n_ring[:, sbase : sbase + U, m - 4, :], src,
                                biasc_sb[:, m : m + 1], None, OP.add,
                            )
                    return f

                pieces[0].append(dma_rt(0))
                pieces[1].append(dma_rt(1))
                pieces[2].append(conv_rt(0))
                pieces[3].append(conv_rt(1))
                for k in range(KH):
                    pieces[4 + k].append(tpose(k))
                # m-major groups, 3 mms/step over s=12..27; copy after stop
                for m in range(M3):
                    s0 = 12 + 8 * m // 3
                    pieces[s0].append(mm(m, (0, 1, 2)))
                    pieces[s0 + 1].append(mm(m, (3, 4, 5)))
                    pieces[s0 + 2].append(mm(m, (6, 7)))
                    pieces[s0 + 2].append(copy_m(m))
                return pieces

            # ================= GRU step =================================
            # Stage-major emission: both chains' same-stage ops are
            # adjacent in each in-order engine queue, so chain 1's work
            # fills chain 0's dependency gaps instead of head-of-line
            # blocking behind it.
            def emit_step(s, cs=(0, 1)):
                pv = (s - 1) % U2
                st = {}
                for c in cs:
                    b0 = c * NBC
                    bs = slice(b0, b0 + NBC)
                    ps = ps_cp[c][s % 2][:, : 6 * NBC].rearrange(
                        "p (g j) -> p g j", g=6
                    )
                    st[c] = (bs, ps)
                    # single group-opening preload: xp r|z + n-gate b_hh
                    nc.tensor.matmul(
                        ps, lhsT=idt_b_sb, rhs=pre_ring[:, s, :, bs],
                        start=True, stop=False,
                    )
                    # W_hh: rz gates first (release the sigmoid), n after
                    for i in range(6):
                        for k in range(KC):
                            nc.tensor.matmul(
                                ps[:, i, :],
                                lhsT=whh_sb[:, k, ts(m := i, 128)],
                                rhs=h_ring[:, k, pv, bs],
                                start=False,
                                stop=(i == 5 and k == KC - 1),
                            )
                srz = {}
                for c in cs:
                    bs, ps = st[c]
                    s_rz = wpool.tile(
                        [128, 4, NBC], F32, tag=f"s_rz{c}", name=f"s_rz{c}"
                    )
                    nc.scalar.activation(s_rz, ps[:, 0:4, :], AF.Sigmoid)
                    srz[c] = s_rz
                m1s = {}
                for c in cs:
                    bs, ps = st[c]
                    m1 = wpool.tile(
                        [128, 2, NBC], F32, tag=f"m1_{c}", name=f"m1_{c}"
                    )
                    nc.vector.tensor_tensor(m1, srz[c][:, 0:2, :],
                                            ps[:, 4:6, :], OP.mult)
                    m1s[c] = m1
                nins = {}
                for c in cs:
                    bs, _ = st[c]
                    nin = wpool.tile(
                        [128, 2, NBC], F32, tag=f"nin{c}", name=f"nin{c}"
                    )
                    e_nin = nc.vector if c == 0 else nc.gpsimd
                    e_nin.tensor_tensor(nin, m1s[c], n_ring[:, s, :, bs],
                                        OP.add)
                    nins[c] = nin
                # q,t2 on gpsimd during the tanh window
                qts = {}
                for c in cs:
                    bs, _ = st[c]
                    hp = h_ring[:, :, pv, bs]
                    q = wpool.tile([128, 2, NBC], F32, tag=f"q_{c}",
                                   name=f"q_{c}")
                    t2 = wpool.tile([128, 2, NBC], F32, tag=f"t2_{c}",
                                    name=f"t2_{c}")
                    nc.gpsimd.tensor_tensor(q, srz[c][:, 2:4, :], hp, OP.mult)
                    nc.gpsimd.tensor_tensor(t2, hp, q, OP.subtract)
                    qts[c] = t2
                nsb = {}
                for c in cs:
                    n_sb = wpool.tile(
                        [128, 2, NBC], F32, tag=f"n_sb{c}", name=f"n_sb{c}"
                    )
                    nc.scalar.activation(n_sb, nins[c], AF.Tanh)
                    nsb[c] = n_sb
                for c in cs:
                    bs, _ = st[c]
                    zn = wpool.tile([128, 2, NBC], F32, tag=f"zn{c}",
                                    name=f"zn{c}")
                    nc.vector.tensor_tensor(zn, srz[c][:, 2:4, :], nsb[c],
                                            OP.mult)
                    nc.vector.tensor_tensor(h_ring[:, :, s, bs], zn, qts[c],
                                            OP.add)

            def flush(rowbase, lo, hi):
                for c2 in range(KC):
                    nc.sync.dma_start(
                        hist[:, c2, ds(rowbase + lo * NB, (hi - lo) * NB)],
                        h_ring[:, c2, lo:hi, :].rearrange("p t b -> p (t b)"),
                    )

            # ================= prologue: project block 0 =================
            for plist in proj_pieces(0, 0):
                for f in plist:
                    f()

            # ================= main loop (2 blocks / iter) ==============
            if n_pairs > 0:
                with tc.For_i(
                    0, n_pairs * U2 * NB, step=U2 * NB, staggered_reset=True,
                    hint_engines=(mybir.EngineType.PE,),
                ) as iv:
                    pieces_b = proj_pieces(iv + U * NB, U)       # block 2k+1
                    pieces_a = proj_pieces(iv + U2 * NB, 0)      # block 2k+2
                    for s in range(U2):
                        emit_step(s)
                        if s < U:
                            for f in pieces_b[s]:
                                f()
                        else:
                            for f in pieces_a[s - U]:
                                f()
                        if s in (17, 33, 49):
                            lo = ((s - 1) // 16 - 1) * 16  # 0, 16, 32
                            flush(iv, lo, lo + 16)
                    flush(iv, U2 - 16, U2)

            # ================= tail blocks ==============================
            done_b = 2 * n_pairs  # gru blocks completed by the loop
            rowbase = done_b * U * NB
            # project remaining blocks done_b+1 .. nblocks-1 (block done_b
            # was projected by the loop's last iteration or the prologue)
            rem_proj = []
            for bkt in range(done_b + 1, nblocks):
                rem_proj.append(proj_pieces(bkt * U * NB, (bkt % 2) * U))
            assert len(rem_proj) <= 1, "tail can only project one block"
            rem_steps = t_total - done_b * U

            for s in range(rem_steps):
                sl = (done_b * U + s) % U2
                emit_step(sl)
                if s < U:
                    for pl in rem_proj:
                        for f in pl[s]:
                            f()
            # flush the tail in 16-step chunks
            sbase0 = (done_b * U) % U2
            off = 0
            while off < rem_steps:
                hi = min(off + 16, rem_steps)
                flush(rowbase, sbase0 + off, sbase0 + hi)
                off = hi

            # close PSUM pools before phase 3
            gp_ctx.__exit__(None, None, None)
            tp_ctx.__exit__(None, None, None)
            pp_ctx.__exit__(None, None, None)

            # ================= phase 3: log_softmax =====================
            # h = tanh(.) is in (-1,1): exp can't overflow, skip max pass.
            with tc.tile_pool(name="p3ps", bufs=2, space="PSUM") as p3pool:
                nblk = (t_total + 15) // 16
                x_all = cpool.tile([128, nblk, C], BF16)
                se_all = cpool.tile([128, nblk], F32)
                s_all = cpool.tile([128, nblk], F32)
                nc.vector.memset(se_all, 1.0)

                blocks = []
                t0 = 0
                b = 0
                while t0 < t_total:
                    bt = min(16, t_total - t0)
                    rows = bt * NB
                    blocks.append((t0, bt, b, rows))
                    hsb = wpool.tile([128, KC, 16, NB], BF16, tag="hsb", bufs=3)
                    nc.sync.dma_start(
                        hsb[:, :, :bt, :].rearrange("p c t b -> p c (t b)"),
                        hist[:, :, ds(t0 * NB, rows)],
                    )
                    ps3 = p3pool.tile([128, 1024], BF16, tag="ps3")
                    for c2 in range(KC):
                        nc.tensor.transpose(
                            ps3[:rows, ts(c2, 128)], hsb[:, c2, :bt, :],
                            idt_b_sb,
                        )
                    nc.vector.tensor_copy(x_all[:rows, b, :], ps3[:rows, :C])
                    escr = wpool.tile([128, C], BF16, tag="escr")
                    nc.scalar.activation(
                        escr[:rows, :], ps3[:rows, :C], AF.Exp,
                        accum_out=se_all[:rows, b : b + 1],
                    )
                    t0 += bt
                    b += 1

                nc.scalar.activation(s_all, se_all, AF.Ln)

                for t0, bt, b, rows in blocks:
                    o_sb = wpool.tile([128, C], F32, tag="o_sb", bufs=3)
                    nc.vector.tensor_scalar(
                        o_sb[:rows, :], x_all[:rows, b, :],
                        s_all[:rows, b : b + 1], None, OP.subtract,
                    )
                    nc.sync.dma_start(out[ds(t0 * NB, rows), :], o_sb[:rows, :])

    nc.compile()
    return nc


def _prep_weights(W_ih, W_hh, b_ih, b_hh):
    bf = ml_dtypes.bfloat16
    # z-gate sign flip: sigmoid(-(xz+hz)) = 1-z from the shared r|z sigmoid
    W_ih = W_ih.copy()
    W_hh = W_hh.copy()
    W_ih[C : 2 * C] *= -1.0
    W_hh[C : 2 * C] *= -1.0

    # lhsT layouts: w[p, k, m*128+q] = W[m*128+q, k*128+p]
    def lhst(W, kc):
        t = W.T.reshape(kc, 128, W.shape[0])  # [k, p, g]
        return np.ascontiguousarray(t.transpose(1, 0, 2)).astype(bf)

    wih = lhst(W_ih, KH)
    whh = lhst(W_hh, KC)
    idt_b = np.eye(128, dtype=bf)
    ball = (b_ih + b_hh).astype(np.float32).copy()
    ball[C : 2 * C] *= -1.0            # z gate: negated sum
    ball[2 * C :] = b_ih[2 * C :]      # n gate: b_ih only
    biasc = np.ascontiguousarray(ball.reshape(M3, 128).T).astype(np.float32)
    # n-gate b_hh broadcast to all U2 ring slots: [p, slot, c2, b]
    bhn_v = b_hh[2 * C :].reshape(2, 128).T.astype(np.float32)  # [p, c2]
    bhnb = np.broadcast_to(
        bhn_v[:, None, :, None], (128, U2, 2, NB)
    ).reshape(128, U2 * 2 * NB).astype(bf)
    bhnb = np.ascontiguousarray(bhnb)
    return wih, whh, idt_b, biasc, bhnb


_CACHED = {}


def _make_in_maps(inputs):
    encoder_output = np.asarray(inputs["encoder_output"], dtype=np.float32)
    W_ih = np.asarray(inputs["W_ih"], dtype=np.float32)
    W_hh = np.asarray(inputs["W_hh"], dtype=np.float32)
    b_ih = np.asarray(inputs["b_ih"], dtype=np.float32)
    b_hh = np.asarray(inputs["b_hh"], dtype=np.float32)
    t_total = encoder_output.shape[0]
    nblocks = (t_total + U - 1) // U
    tpad = nblocks * U
    wih, whh, idt_b, biasc, bhnb = _prep_weights(W_ih, W_hh, b_ih, b_hh)
    in_maps = []
    for c in range(NCORES):
        shard = encoder_output[:, c * NB : (c + 1) * NB, :]  # [T, NB, H]
        enc = np.zeros((tpad * NB, H), dtype=np.float32)
        enc[: t_total * NB] = shard.reshape(t_total * NB, H)
        in_maps.append(
            {
                "enc": enc, "wih": wih, "whh": whh, "idt_b": idt_b,
                "biasc": biasc, "bhnb": bhnb,
            }
        )
    return in_maps


def kernel(encoder_output, W_ih, W_hh, b_ih, b_hh):
    encoder_output = np.asarray(encoder_output, dtype=np.float32)

    t_total = encoder_output.shape[0]
    if "nc" not in _CACHED or _CACHED.get("t") != t_total:
        _CACHED["nc"] = build_gru_nc(t_total)
        _CACHED["t"] = t_total

    in_maps = _make_in_maps(
        {
            "encoder_output": encoder_output,
            "W_ih": W_ih, "W_hh": W_hh, "b_ih": b_ih, "b_hh": b_hh,
        }
    )

    res = run_bass_kernel_spmd(_CACHED["nc"], in_maps, list(range(NCORES)))
    outs = [
        res.results[c]["out"].reshape(t_total, NB, C) for c in range(NCORES)
    ]
    return np.concatenate(outs, axis=1)


if __name__ == "__main__":
    # quick smoke test with small T
    t_small = int(sys.argv[1]) if len(sys.argv) > 1 else 72
    rng = np.random.default_rng(0)
    enc = rng.standard_normal((t_small, N, H), dtype=np.float32)
    s = 0.05
    Wih = rng.standard_normal((G3, H), dtype=np.float32) * s
    Whh = rng.standard_normal((G3, C), dtype=np.float32) * s
    bih = rng.standard_normal(G3).astype(np.float32) * s
    bhh = rng.standard_normal(G3).astype(np.float32) * s

    got = kernel(enc, Wih, Whh, bih, bhh)

    xp = enc.reshape(-1, H) @ Wih.T + bih
    xp = xp.reshape(t_small, N, G3)
    h = np.zeros((N, C), dtype=np.float32)
    outs = []
    sig = lambda x: 1.0 / (1.0 + np.exp(-x))
    for t in range(t_small):
        gh = h @ Whh.T + bhh
        xr, xz, xn = np.split(xp[t], 3, axis=-1)
        hr, hz, hn = np.split(gh, 3, axis=-1)
        r = sig(xr + hr)
        z = sig(xz + hz)
        n = np.tanh(xn + r * hn)
        h = (1.0 - z) * n + z * h
        outs.append(h.copy())
    ref = np.stack(outs)
    mx = ref.max(-1, keepdims=True)
    ref = ref - mx - np.log(np.exp(ref - mx).sum(-1, keepdims=True))

    err = np.abs(got - ref)
    print("abs err max:", err.max(), " rel:", err.max() / np.abs(ref).max())


# revision 24
# speedup vs baseline: 1.0619x; 1.0613x over previous
"""GRU decoder kernel for Trainium2 (Bass/Tile), data-parallel over batch N.

Problem: T=1000, N=64, H=1024, C=256 GRU with batched input projection and
log_softmax output.  Each of the 8 cores handles N/8 = 8 batch rows:
  phase 1: xp = enc @ W_ih.T + biases   (bf16 matmul, PE-transposed enc)
  phase 2: 1000 sequential GRU steps    (bf16 W_hh stationary, h moving)
  phase 3: log_softmax over C           (PE transpose + Exp/Ln on ACT)

All phase-2 stream DMAs (xp ring refills, hist flushes) use layouts that
collapse to contiguous per-partition runs so descriptor counts stay low.
"""

import sys

for _p in ("/opt/trn_rl_repo",):
    if _p not in sys.path:
        sys.path.insert(0, _p)

import numpy as np
import ml_dtypes

import concourse.bass as bass
import concourse.bacc as bacc
import concourse.mybir as mybir
import concourse.tile as tile
from concourse.bass import ds, ts
from concourse.bass_utils import run_bass_kernel_spmd

F32 = mybir.dt.float32
BF16 = mybir.dt.bfloat16
AF = mybir.ActivationFunctionType
OP = mybir.AluOpType

T, N, H, C = 1000, 64, 1024, 256
G3 = 3 * C  # 768
NCORES = 8
NB = N // NCORES  # 8 batch rows per core
KH = H // 128     # 8 k-chunks for projection
M3 = G3 // 128    # 6 m-chunks of gate dim
KC = C // 128     # 2 k-chunks for recurrence
import os as _os
U = int(_os.environ.get("GRU_U", "32"))  # steps per For_i iteration
UH = U // 2


def build_gru_nc(t_total=T):
    """Build the Bass program. t_total must be = 8 (mod 16) or multiple of 16."""
    nc = bacc.Bacc(None, target_bir_lowering=False)

    rem = t_total % U
    n_iters = t_total // U
    assert rem == 0 or rem <= UH, f"t_total={t_total} bad for U={U}"

    # ---- parameters -----------------------------------------------------
    enc = nc.declare_dram_parameter("enc", [t_total * NB, H], F32, isOutput=False)
    # W_ih as lhsT tiles: wih[p, k, m*128+q] = W_ih[m*128+q, k*128+p]
    wih = nc.declare_dram_parameter("wih", [128, KH, G3], BF16, isOutput=False)
    # W_hh as lhsT tiles: whh[p, k, m*128+q] = W_hh[m*128+q, k*128+p]
    whh = nc.declare_dram_parameter("whh", [128, KC, G3], BF16, isOutput=False)
    idt_b = nc.declare_dram_parameter("idt_b", [128, 128], BF16, isOutput=False)
    idt_f = nc.declare_dram_parameter("idt_f", [128, 128], F32, isOutput=False)
    # bias column m: rz chunks get b_ih+b_hh, n chunks get b_ih only
    biasc = nc.declare_dram_parameter("biasc", [128, M3], F32, isOutput=False)
    # b_hh for the n gate, broadcast over batch: bhn[p, c2*8+j] = b_hh[2C+c2*128+p]
    bhn = nc.declare_dram_parameter("bhn", [128, 2 * NB], BF16, isOutput=False)
    out = nc.declare_dram_parameter("out", [t_total * NB, C], F32, isOutput=True)

    # ---- DRAM scratch (t-major, matching the SBUF ring layouts) ---------
    # pad t by UH: the lookahead refill of the last iteration overshoots
    xp_rz = nc.dram_tensor("xp_rz", [128, t_total + UH, 4, NB], BF16)
    xp_n = nc.dram_tensor("xp_n", [128, t_total + UH, 2, NB], F32)
    hist = nc.dram_tensor("hist", [128, 2, t_total, NB], F32)

    with tile.TileContext(nc) as tc:
        with (
            tc.tile_pool(name="const", bufs=1) as cpool,
            tc.tile_pool(name="work", bufs=2) as wpool,
        ):
            idt_b_sb = cpool.tile([128, 128], BF16)
            nc.sync.dma_start(idt_b_sb, idt_b[:, :])
            idt_f_sb = cpool.tile([128, 128], F32)
            nc.sync.dma_start(idt_f_sb, idt_f[:, :])
            wih_sb = cpool.tile([128, KH, G3], BF16)
            nc.sync.dma_start(wih_sb, wih[:, :, :])
            whh_sb = cpool.tile([128, KC, G3], BF16)
            nc.sync.dma_start(whh_sb, whh[:, :, :])
            biasc_sb = cpool.tile([128, M3], F32)
            nc.sync.dma_start(biasc_sb, biasc[:, :])
            bhn_sb = cpool.tile([128, 2 * NB], BF16)
            nc.sync.dma_start(bhn_sb, bhn[:, :])

            # ================= phase 1: input projection ================
            p1 = tc.tile_pool(name="p1psum", bufs=1, space="PSUM")
            pspool = p1.__enter__()
            p1t = tc.tile_pool(name="p1tpose", bufs=2, space="PSUM")
            ptpool = p1t.__enter__()
            t0 = 0
            while t0 < t_total:
                bt = min(64, t_total - t0)
                rows = bt * NB
                ntile = (rows + 127) // 128

                encT = wpool.tile([128, KH, 512], BF16, tag="encT")
                for ti in range(ntile):
                    r0 = t0 * NB + ti * 128
                    rr = min(128, t_total * NB - r0)
                    enc_sb = wpool.tile([128, H], F32, tag="enc_in", bufs=3)
                    nc.sync.dma_start(enc_sb[:rr, :], enc[ds(r0, rr), :])
                    enc_bf = wpool.tile([128, H], BF16, tag="enc_bf", bufs=2)
                    nc.scalar.activation(enc_bf[:rr, :], enc_sb[:rr, :], AF.Copy)
                    for k in range(KH):
                        ps_t = ptpool.tile([128, 128], BF16, tag="tpose")
                        nc.tensor.transpose(
                            ps_t[:, :rr], enc_bf[:rr, ts(k, 128)],
                            idt_b_sb[:rr, :rr],
                        )
                        nc.vector.tensor_copy(
                            encT[:, k, ds(ti * 128, rr)], ps_t[:, :rr]
                        )

                psm = [
                    pspool.tile(
                        [128, 512], F32, tag=f"pj_psum{m}", name=f"pj_psum{m}"
                    )
                    for m in range(M3)
                ]
                for k in range(KH):
                    for m in range(M3):
                        nc.tensor.matmul(
                            psm[m][:, :rows],
                            lhsT=wih_sb[:, k, ts(m, 128)],
                            rhs=encT[:, k, :rows],
                            start=(k == 0),
                            stop=(k == KH - 1),
                        )
                # stage into ring-layout tiles so the DRAM DMAs are contiguous
                st_rz = wpool.tile([128, 64, 4, NB], BF16, tag="st_rz", bufs=2)
                for m in range(4):
                    nc.vector.tensor_scalar(
                        st_rz[:, :bt, m, :],
                        psm[m][:, :rows].rearrange("p (t j) -> p t j", j=NB),
                        biasc_sb[:, m : m + 1], None, OP.add,
                    )
                nc.sync.dma_start(
                    xp_rz[:, ds(t0, bt), :, :], st_rz[:, :bt, :, :]
                )
                st_n = wpool.tile([128, 64, 2, NB], F32, tag="st_n", bufs=2)
                for m in (4, 5):
                    nc.vector.tensor_scalar(
                        st_n[:, :bt, m - 4, :],
                        psm[m][:, :rows].rearrange("p (t j) -> p t j", j=NB),
                        biasc_sb[:, m : m + 1], None, OP.add,
                    )
                nc.sync.dma_start(
                    xp_n[:, ds(t0, bt), :, :], st_n[:, :bt, :, :]
                )
                t0 += bt

            # zero the lookahead pad so the overshooting refill reads clean
            zpad = wpool.tile([128, UH, 4, NB], BF16, tag="zpad")
            nc.vector.memset(zpad, 0.0)
            nc.sync.dma_start(xp_rz[:, t_total : t_total + UH, :, :], zpad)
            zpad_n = wpool.tile([128, UH, 2, NB], F32, tag="zpad_n")
            nc.vector.memset(zpad_n, 0.0)
            nc.sync.dma_start(xp_n[:, t_total : t_total + UH, :, :], zpad_n)

            p1t.__exit__(None, None, None)
            p1.__exit__(None, None, None)

            # ================= phase 2: GRU recurrence ==================
            p2 = tc.tile_pool(name="p2psum", bufs=2, space="PSUM")
            ptpool = p2.__enter__()
            rz_ring = cpool.tile([128, U, 4, NB], BF16)
            n_ring = cpool.tile([128, U, 2, NB], F32)
            h_ring = cpool.tile([128, 2, U, NB], F32)   # fp32 h (c2-major)
            h_bf = cpool.tile([128, 2, KC * NB], BF16)  # ping-pong bf16 h

            nc.vector.memset(h_bf[:, :, :], 0.0)
            nc.gpsimd.memset(h_ring[:, :, U - 1, :], 0.0)

            # pre-warm the sigmoid/tanh table set so the body's table load
            # hoists out of the loop
            warm = wpool.tile([1, 1], F32, tag="warm")
            nc.scalar.activation(warm, bhn_sb[:1, :1], AF.Sigmoid)

            def emit_step(s_glob, slot):
                """One GRU step reading xp rings at `slot`, h from slot-1."""
                pv = (slot - 1) % U
                hb_in = h_bf[:, (s_glob + 1) % 2, :]
                hb_out = h_bf[:, s_glob % 2, :]
                hp3 = h_ring[:, :, pv, :]  # [128, 2, NB]
                v3 = lambda ap: ap.rearrange("p (c j) -> p c j", c=2)

                ps_r = ptpool.tile([128, 2 * NB], F32, tag="ps_r")
                ps_z = ptpool.tile([128, 2 * NB], F32, tag="ps_z")
                ps_n = ptpool.tile([128, 2 * NB], F32, tag="ps_n")

                # xp / bias preloads (independent of h): one group per tile
                nc.tensor.matmul(
                    ps_r, lhsT=idt_b_sb, rhs=rz_ring[:, slot, 0:2, :],
                    start=True, stop=False,
                )
                nc.tensor.matmul(
                    ps_z, lhsT=idt_b_sb, rhs=rz_ring[:, slot, 2:4, :],
                    start=True, stop=False,
                )
                nc.tensor.matmul(
                    ps_n, lhsT=idt_b_sb, rhs=bhn_sb, start=True, stop=False,
                )

                # W_hh matmuls: r chunks, then n, then z.  stop only on the
                # very last matmul touching each psum tile (zero-region rule).
                def wmm(ps, m, col, last):
                    for k in range(KC):
                        nc.tensor.matmul(
                            ps[:, ts(col, NB)],
                            lhsT=whh_sb[:, k, ts(m, 128)],
                            rhs=hb_in[:, ts(k, NB)],
                            start=False,
                            stop=(last and k == KC - 1),
                        )

                for ps, ms in ((ps_r, (0, 1)), (ps_n, (4, 5)), (ps_z, (2, 3))):
                    for c2, m in enumerate(ms):
                        wmm(ps, m, c2, c2 == 1)

                r_sb = wpool.tile([128, 2 * NB], F32, tag="r_sb")
                nc.scalar.activation(r_sb, ps_r, AF.Sigmoid)
                zb_sb = wpool.tile([128, 2 * NB], F32, tag="zb_sb")
                nc.scalar.activation(zb_sb, ps_z, AF.Sigmoid, scale=-1.0)

                m1 = wpool.tile([128, 2 * NB], F32, tag="m1")
                nc.vector.tensor_tensor(m1, r_sb, ps_n, OP.mult)
                nin = wpool.tile([128, 2 * NB], F32, tag="nin")
                nc.vector.tensor_tensor(
                    v3(nin), v3(m1), n_ring[:, slot, :, :], OP.add,
                )
                n_sb = wpool.tile([128, 2 * NB], F32, tag="n_sb")
                nc.scalar.activation(n_sb, nin, AF.Tanh)

                # q/t2 on vector during the tanh window: zb_sb and hp3 are
                # ready before tanh, and keeping them on vector makes the
                # zn -> hb_out chain same-engine (no cross-engine sem gap)
                q_sb = wpool.tile([128, 2 * NB], F32, tag="q_sb")
                nc.vector.tensor_tensor(v3(q_sb), v3(zb_sb), hp3, OP.mult)
                t2 = wpool.tile([128, 2 * NB], F32, tag="t2")
                nc.vector.tensor_tensor(v3(t2), hp3, v3(q_sb), OP.subtract)
                zn = wpool.tile([128, 2 * NB], F32, tag="zn")
                nc.vector.tensor_tensor(zn, zb_sb, n_sb, OP.mult)
                nc.vector.tensor_tensor(hb_out, zn, t2, OP.add)
                nc.gpsimd.tensor_tensor(
                    h_ring[:, :, slot, :], v3(zn), v3(t2), OP.add,
                )

            def refill(iv, lo, hi):
                nc.sync.dma_start(
                    rz_ring[:, lo:hi, :, :], xp_rz[:, ds(iv, hi - lo), :, :]
                )
                nc.sync.dma_start(
                    n_ring[:, lo:hi, :, :], xp_n[:, ds(iv, hi - lo), :, :]
                )

            def flush(iv, lo, hi):
                for c2 in range(KC):
                    nc.sync.dma_start(
                        hist[:, c2, ds(iv, hi - lo), :],
                        h_ring[:, c2, lo:hi, :],
                    )

            refill(0, 0, UH)  # prologue: slots 0..7 <- t 0..7
            if n_iters > 0:
                with tc.For_i(
                    0, n_iters * U, step=U, staggered_reset=True,
                    hint_engines=(mybir.EngineType.PE,),
                ) as iv:
                    refill(iv + UH, UH, U)
                    for s in range(UH):
                        emit_step(s, s)
                    flush(iv, 0, UH)
                    refill(iv + U, 0, UH)
                    for s in range(UH, U):
                        emit_step(s, s)
                    flush(iv + UH, UH, U)
            if rem:
                base = n_iters * U
                for s in range(rem):
                    emit_step(s, s)
                flush(base, 0, rem)

            p2.__exit__(None, None, None)

            # ================= phase 3: log_softmax =====================
            p3 = tc.tile_pool(name="p3psum", bufs=2, space="PSUM")
            ptpool = p3.__enter__()
            nblk = (t_total + 15) // 16
            se_all = cpool.tile([128, nblk], F32)
            mx_all = cpool.tile([128, nblk], F32)
            nc.vector.memset(se_all, 1.0)
            nc.vector.memset(mx_all, 0.0)

            def p3_transpose(t0, bt):
                rows = bt * NB
                hsb = wpool.tile([128, 2, 16, NB], F32, tag="hsb", bufs=3)
                nc.sync.dma_start(hsb[:, :, :bt, :], hist[:, :, ds(t0, bt), :])
                ps3 = ptpool.tile([128, 256], F32, tag="ps3")
                for c2 in range(KC):
                    nc.tensor.transpose(
                        ps3[:rows, ts(c2, 128)], hsb[:, c2, :bt, :], idt_f_sb
                    )
                return ps3, rows

            # pass 1: max + sum(exp(x-max)) per block (Exp table only)
            blocks = []
            t0 = 0
            b = 0
            while t0 < t_total:
                bt = min(16, t_total - t0)
                blocks.append((t0, bt, b))
                ps3, rows = p3_transpose(t0, bt)
                nc.vector.tensor_reduce(
                    mx_all[:rows, b : b + 1], ps3[:rows, :],
                    mybir.AxisListType.X, OP.max,
                )
                ngm = wpool.tile([128, 1], F32, tag="ngm")
                nc.vector.tensor_scalar_mul(
                    ngm[:rows, :], mx_all[:rows, b : b + 1], -1.0
                )
                escr = wpool.tile([128, 256], BF16, tag="escr")
                nc.scalar.activation(
                    escr[:rows, :], ps3[:rows, :], AF.Exp,
                    bias=ngm[:rows, :], accum_out=se_all[:rows, b : b + 1],
                )
                t0 += bt
                b += 1

            # one Ln over all blocks, then s = mx + ln(se)
            lz_all = cpool.tile([128, nblk], F32)
            nc.scalar.activation(lz_all, se_all, AF.Ln)
            s_all = cpool.tile([128, nblk], F32)
            nc.vector.tensor_tensor(s_all, mx_all, lz_all, OP.add)

            # pass 2: out = x - s
            for t0, bt, b in blocks:
                ps3, rows = p3_transpose(t0, bt)
                o_sb = wpool.tile([128, 256], F32, tag="o_sb", bufs=3)
                nc.vector.tensor_scalar(
                    o_sb[:rows, :], ps3[:rows, :], s_all[:rows, b : b + 1],
                    None, OP.subtract,
                )
                nc.sync.dma_start(out[ds(t0 * NB, rows), :], o_sb[:rows, :])
            p3.__exit__(None, None, None)

    nc.compile()
    return nc


def _prep_weights(W_ih, W_hh, b_ih, b_hh):
    bf = ml_dtypes.bfloat16
    # lhsT layouts: w[p, k, m*128+q] = W[m*128+q, k*128+p]
    def lhst(W, kc):
        t = W.T.reshape(kc, 128, W.shape[0])  # [k, p, g]
        return np.ascontiguousarray(t.transpose(1, 0, 2)).astype(bf)

    wih = lhst(W_ih, KH)
    whh = lhst(W_hh, KC)
    idt_b = np.eye(128, dtype=bf)
    idt_f = np.eye(128, dtype=np.float32)
    ball = (b_ih + b_hh).astype(np.float32).copy()
    ball[2 * C :] = b_ih[2 * C :]  # n gate: b_ih only (b_hh_n goes inside r*)
    biasc = np.ascontiguousarray(ball.reshape(M3, 128).T).astype(np.float32)
    bhn_v = b_hh[2 * C :].reshape(2, 128).T  # [p, c2]
    bhn = np.repeat(bhn_v[:, :, None], NB, axis=2).reshape(128, 2 * NB).astype(bf)
    return wih, whh, idt_b, idt_f, biasc, bhn


_CACHED = {}


def _make_in_maps(inputs):
    encoder_output = np.asarray(inputs["encoder_output"], dtype=np.float32)
    W_ih = np.asarray(inputs["W_ih"], dtype=np.float32)
    W_hh = np.asarray(inputs["W_hh"], dtype=np.float32)
    b_ih = np.asarray(inputs["b_ih"], dtype=np.float32)
    b_hh = np.asarray(inputs["b_hh"], dtype=np.float32)
    t_total = encoder_output.shape[0]
    wih, whh, idt_b, idt_f, biasc, bhn = _prep_weights(W_ih, W_hh, b_ih, b_hh)
    in_maps = []
    for c in range(NCORES):
        shard = encoder_output[:, c * NB : (c + 1) * NB, :]  # [T, NB, H]
        in_maps.append(
            {
                "enc": np.ascontiguousarray(shard.reshape(t_total * NB, H)),
                "wih": wih, "whh": whh, "idt_b": idt_b, "idt_f": idt_f,
                "biasc": biasc, "bhn": bhn,
            }
        )
    return in_maps


def kernel(encoder_output, W_ih, W_hh, b_ih, b_hh):
    encoder_output = np.asarray(encoder_output, dtype=np.float32)

    t_total = encoder_output.shape[0]
    if "nc" not in _CACHED or _CACHED.get("t") != t_total:
        _CACHED["nc"] = build_gru_nc(t_total)
        _CACHED["t"] = t_total

    in_maps = _make_in_maps(
        {
            "encoder_output": encoder_output,
            "W_ih": W_ih, "W_hh": W_hh, "b_ih": b_ih, "b_hh": b_hh,
        }
    )

    res = run_bass_kernel_spmd(_CACHED["nc"], in_maps, list(range(NCORES)))
    outs = [
        res.results[c]["out"].reshape(t_total, NB, C) for c in range(NCORES)
    ]
    return np.concatenate(outs, axis=1)


if __name__ == "__main__":
    # quick smoke test with small T
    t_small = 24
    rng = np.random.default_rng(0)
    enc = rng.standard_normal((t_small, N, H), dtype=np.float32)
    s = 0.05
    Wih = rng.standard_normal((G3, H), dtype=np.float32) * s
    Whh = rng.standard_normal((G3, C), dtype=np.float32) * s
    bih = rng.standard_normal(G3).astype(np.float32) * s
    bhh = rng.standard_normal(G3).astype(np.float32) * s

    got = kernel(enc, Wih, Whh, bih, bhh)

    xp = enc.reshape(-1, H) @ Wih.T + bih
    xp = xp.reshape(t_small, N, G3)
    h = np.zeros((N, C), dtype=np.float32)
    outs = []
    sig = lambda x: 1.0 / (1.0 + np.exp(-x))
    for t in range(t_small):
        gh = h @ Whh.T + bhh
        xr, xz, xn = np.split(xp[t], 3, axis=-1)
        hr, hz, hn = np.split(gh, 3, axis=-1)
        r = sig(xr + hr)
        z = sig(xz + hz)
        n = np.tanh(xn + r * hn)
        h = (1.0 - z) * n + z * h
        outs.append(h.copy())
    ref = np.stack(outs)
    mx = ref.max(-1, keepdims=True)
    ref = ref - mx - np.log(np.exp(ref - mx).sum(-1, keepdims=True))

    err = np.abs(got - ref)
    print("abs err max:", err.max(), " rel:", err.max() / np.abs(ref).max())

